# revision 53
# baseline (speedup 1.0000x reference)
"""AttentionBlock3D (GroupNorm + single-head self-attention + residual) on 8 TRN2 cores.

Sharding: core = (batch b in {0,1}) x (1024-row slice of the 4096 attention rows).
Each core computes its batch's GroupNorm stats (cheap, on the PE) and
attention + output projection for its own 1024 query rows. No collectives.
The host ROTATES each core's x copy so that its query rows are always
columns 0..1024 (attention is permutation-invariant over keys).

fp8 DoubleRow pipeline with K and V eliminated:
  - x ships twice in fp8 e4m3: xb8 [128, 2, N] (channel-major) and
    xt8 [128, 32, 258] (position-major with built-in ones columns).
  - GroupNorm stats on the PE: x^T x accumulated per channel half; the ones
    column gives Sum x, the diagonal (mask reduce) gives Sum x^2.
  - S = hn^T G hn_q with G = Wk^T Wq (host): q' = (G.A-folded)^T x_q scaled
    by A, S = x^T q' via DoubleRow with xb8 stationary — no K tensor.
    Per-key bias delta[m] = (A o (G B + Wk^T bq))^T x[:,m] enters S as one
    extra DoubleRow matmul with constant-column rhs (coef broadcast).
  - u = x^T E via DoubleRow with xt8 stationary — no V tensor; the output
    projection fuses Pv = Wp @ Wv (host) with A folded on device:
    out = x_q + bpe + (Pv.A)^T u / r,  bpe = bp + Wp cv, cv = Wv B + bv.
  - E = exp(S/16 - 2) on ACT in [128,1024] tiles (e^-2 cancels against r).
    ACT exp is the bottleneck engine (~4.2M elements/core).
"""

import os
import numpy as np
from contextlib import ExitStack

C = 256          # channels
N = 4096         # spatial positions (16*16*16)
NQ = 1024        # query rows per core
GROUPS = 8
GSIZE = C // GROUPS
EPS = 1e-5

_CACHE = {}
LAST_RESULTS = None  # test harness can inspect trace results


def _build_nc():
    import concourse.bacc as bacc
    import concourse.tile as tile
    from concourse import mybir

    f32 = mybir.dt.float32
    f32r = mybir.dt.float32r
    bf16 = mybir.dt.bfloat16
    f8 = mybir.dt.float8e4
    AF = mybir.ActivationFunctionType
    DR = mybir.MatmulPerfMode.DoubleRow
    ADD = mybir.AluOpType.add
    MULT = mybir.AluOpType.mult

    nc = bacc.Bacc("TRN2", target_bir_lowering=False, debug=False,
                   enable_asserts=False)

    # ---- DRAM I/O (per-core) ----
    xt8_d = nc.dram_tensor("xt8", [128, 32 * 272], f8, kind="ExternalInput").ap()
    xb8_d = nc.dram_tensor("xb8", [128, 2 * N], f8, kind="ExternalInput").ap()
    gt_d = nc.dram_tensor("gt", [128, 2 * C], bf16, kind="ExternalInput").ap()
    pvt_d = nc.dram_tensor("pvt", [128, 2 * C], bf16, kind="ExternalInput").ap()
    wall_d = nc.dram_tensor("wall", [128, 2 * 2 * C], bf16, kind="ExternalInput").ap()
    small_d = nc.dram_tensor("small", [128, 26], f32, kind="ExternalInput").ap()
    gmask8_d = nc.dram_tensor("gmask8", [GROUPS, C], f32, kind="ExternalInput").ap()
    imask_d = nc.dram_tensor("imask", [128, 128], bf16, kind="ExternalInput").ap()
    dxq_d = nc.dram_tensor("dxq", [128, 2 * NQ], bf16, kind="ExternalInput").ap()
    out_d = nc.dram_tensor("out", [128, 2 * NQ], bf16, kind="ExternalOutput").ap()

    with tile.TileContext(nc) as tc, ExitStack() as ctx:
        big = ctx.enter_context(tc.tile_pool(name="big", bufs=1))
        consts = ctx.enter_context(tc.tile_pool(name="consts", bufs=1))
        work = ctx.enter_context(tc.tile_pool(name="work", bufs=4))
        epool = ctx.enter_context(tc.tile_pool(name="epool", bufs=5))
        # PSUM banks: psp 2x2 + pacc 2 + pr 1 + pw 1 = 8
        psp = ctx.enter_context(tc.tile_pool(name="psp", bufs=2, space="PSUM"))
        pacc = ctx.enter_context(tc.tile_pool(name="pacc", bufs=2, space="PSUM"))
        pr = ctx.enter_context(tc.tile_pool(name="pr", bufs=1, space="PSUM"))
        pw = ctx.enter_context(tc.tile_pool(name="pw", bufs=1, space="PSUM"))

        # ---- constants (before the big loads) ----
        # r-matmul stationary = 1/64 so the reciprocal yields 64/r; the /64
        # is repaid in the epilogue stt. Keeps u8 = 64*u/r in fp8's sweet spot.
        ones8 = consts.tile([128, 2, 128], f8)
        nc.vector.memset(ones8, 1.0 / 64.0)
        nbias = consts.tile([128, 1], f32)
        nc.vector.memset(nbias, -3.5)
        # eps8 = Sqrt(EPS^2) on ACT: forces the Sqrt act-table load at t~0
        eps_sq = consts.tile([GROUPS, 1], f32)
        nc.vector.memset(eps_sq, EPS * EPS)
        eps8 = consts.tile([GROUPS, 1], f32)
        nc.scalar.activation(out=eps8, in_=eps_sq, func=AF.Sqrt, scale=1.0)

        # ---- load xt8 (stats + u path) chunked; stats matmuls interleave ----
        xt8 = big.tile([128, 32, 272], f8)
        for ch in range(4):
            nc.sync.dma_start(out=xt8[:, 8 * ch:8 * ch + 8, :],
                              in_=xt8_d[:, 8 * 272 * ch:8 * 272 * (ch + 1)])
        # xx accumulates x^T x per channel half ([128,129]: 128 cols of the
        # x^T x block + ones column giving Sum x), split into two position
        # halves so the first extraction overlaps the remaining xt8 DMA.
        def emit_xx01():
            tiles = []
            for h in range(2):
                t = psp.tile([128, 129], f32, name=f"xx{h}", tag="sp")
                tiles.append(t)
            for s in range(16):
                for h in range(2):
                    nc.tensor.matmul(
                        tiles[h],
                        lhsT=xt8[:, 2 * s:2 * s + 2, 136 * h:136 * h + 128],
                        rhs=xt8[:, 2 * s:2 * s + 2, 136 * h:136 * h + 129],
                        start=(s == 0), stop=(s == 15), perf_mode=DR)
            return tiles

        # ---- smalls + G^T + xb8 query cols (early, for q') ----
        small_sb = consts.tile([128, 26], f32)
        nc.sync.dma_start(out=small_sb, in_=small_d)
        imask = consts.tile([128, 128], bf16)
        nc.sync.dma_start(out=imask, in_=imask_d)
        gmask8 = consts.tile([GROUPS, C], f32)
        nc.sync.dma_start(out=gmask8, in_=gmask8_d)
        gamma2 = small_sb[:, 0:2]
        beta2 = small_sb[:, 2:4]
        bv2 = small_sb[:, 4:6]
        wtld2 = small_sb[:, 6:8]          # Wk^T bq (host)
        bp2 = small_sb[:, 8:10]
        gmaskT = [small_sb[:, 10 + 8 * i:18 + 8 * i] for i in range(2)]  # pre-scaled 1/(32N)
        gt = consts.tile([128, 2, C], bf16)
        nc.sync.dma_start(out=gt, in_=gt_d)
        xb8 = big.tile([128, 2, N], f8)
        for i in range(2):
            nc.sync.dma_start(out=xb8[:, i, 0:NQ], in_=xb8_d[:, N * i:N * i + NQ])

        # ---- group stats -> per-channel A (f32) and B (bf16) ----
        # Sum x^2 = diag(x^T x): mask out the diagonal, then column-sum it
        # back to [128,1] with a N=1 matmul (diag matrix -> col sums = diag).
        ones_col = consts.tile([128, 2], f32)
        nc.vector.memset(ones_col, 1.0)
        ones_colr = consts.tile([128, 2], f32r)
        nc.vector.tensor_copy(ones_colr, ones_col)
        stile = work.tile([128, 2, 2], f32, name="stile")  # [:, h, (sx, sxx)]
        xx = emit_xx01()
        for h in range(2):
            nc.vector.tensor_copy(stile[:, h, 0:1], xx[h][:, 128:129])
            scr = work.tile([128, 128], f32r, name="scr", tag="scr", bufs=2)
            nc.vector.tensor_mul(scr, xx[h][:, 0:128], imask)
            sxpool = pw if h == 0 else pr
            sxp = sxpool.tile([128, 2], f32, name="sxp", tag="pw" if h == 0 else "r")
            nc.tensor.matmul(sxp, lhsT=scr, rhs=ones_colr, start=True, stop=True)
            nc.vector.tensor_copy(stile[:, h, 1:2], sxp[:, 0:1])
        gp = pacc.tile([GROUPS, 2], f32, tag="u")
        for h in range(2):
            nc.tensor.matmul(gp, lhsT=gmaskT[h], rhs=stile[:, h, :],
                             start=(h == 0), stop=(h == 1))
        # gmaskT is host-scaled by 1/(32N), so gp = (mean, E[x^2]) directly.
        gsb = work.tile([GROUPS, 2], f32, name="gsb")
        nc.vector.tensor_copy(gsb, gp)
        negvar = work.tile([GROUPS, 1], f32, name="negvar")
        nc.vector.scalar_tensor_tensor(out=negvar, in0=gsb[:, 0:1],
                                       scalar=gsb[:, 0:1], in1=gsb[:, 1:2],
                                       op0=MULT, op1=mybir.AluOpType.subtract)
        gsd = work.tile([GROUPS, 1], f32, name="gsd")
        nc.scalar.activation(out=gsd, in_=negvar, func=AF.Sqrt, bias=eps8,
                             scale=-1.0)
        # preload the Exp act table now (the Sqrt above was ACT's last
        # non-Exp op; loading here keeps the main loop table-stable)
        dummye = consts.tile([1, 1], f32)
        nc.scalar.activation(out=dummye, in_=gsd[0:1, :], func=AF.Exp,
                             scale=1.0, bias=nbias[0:1, :])
        nc.vector.reciprocal(out=gsb[:, 1:2], in_=gsd)

        # B2 holds -B = mean*A - beta (sign fixed up at the consumers)
        A2 = consts.tile([128, 2], f32)
        B2 = consts.tile([128, 2], bf16)
        B2f = work.tile([128, 2], f32, name="B2f")
        for i in range(2):
            gbp = pw.tile([128, 2], f32, name="gbp", tag="pw")
            nc.tensor.matmul(gbp, lhsT=gmask8[:, 128 * i:128 * (i + 1)],
                             rhs=gsb, start=True, stop=True)
            nc.vector.tensor_mul(A2[:, i:i + 1], gamma2[:, i:i + 1], gbp[:, 1:2])
            nc.vector.scalar_tensor_tensor(out=B2f[:, i:i + 1], in0=gbp[:, 0:1],
                                           scalar=A2[:, i:i + 1],
                                           in1=beta2[:, i:i + 1], op0=MULT,
                                           op1=mybir.AluOpType.subtract)
        nc.vector.tensor_copy(B2, B2f)

        # ---- delta coefficient: coef = A o (G B + Wk^T bq) ----
        # Folded into the q' drain: q8 = A*qp + coef makes the single
        # S matmul compute S + delta[m] directly (delta const over n).
        gbv = pacc.tile([128, 2], f32, name="gbv", tag="u")
        for ch in range(2):
            for i in range(2):
                nc.tensor.matmul(gbv[:, ch:ch + 1],
                                 lhsT=gt[:, i, ch * 128:(ch + 1) * 128],
                                 rhs=B2[:, i:i + 1], start=(i == 0), stop=(i == 1))
        coef = work.tile([128, 2], f32, name="coef")
        nc.vector.tensor_sub(coef, wtld2, gbv)     # gbv = -G B
        nc.vector.tensor_mul(coef, coef, A2)

        # ---- fold A into G^T -> fp8 (contraction-side fold for q') ----
        ga8 = consts.tile([128, 2, C], f8)
        for i in range(2):
            nc.vector.tensor_scalar_mul(ga8[:, i, :], gt[:, i, :], A2[:, i:i + 1])

        # ---- q'[c, n] = A[c] * sum_c' (G[c,c'] A[c']) x_q[c', n] + coef[c] ----
        q8 = big.tile([128, 2, NQ], f8)

        def emit_q(qc, chans=(0, 1)):
            for ch in chans:
                ns = slice(qc * 512, (qc + 1) * 512)
                if qc == 0:
                    qp = psp.tile([128, 512], f32, name="qp", tag="sp")
                else:
                    qp = pw.tile([128, 512], f32, name="qp", tag="pw")
                nc.tensor.matmul(qp, lhsT=ga8[:, :, ch * 128:(ch + 1) * 128],
                                 rhs=xb8[:, :, ns], start=True, stop=True,
                                 perf_mode=DR)
                if qc == 0 and ch == 1:
                    # ACT is idle pre-stream; halves the head's drain chain
                    nc.scalar.activation(out=q8[:, ch, ns], in_=qp,
                                         func=AF.Identity,
                                         scale=A2[:, ch:ch + 1],
                                         bias=coef[:, ch:ch + 1])
                else:
                    nc.vector.tensor_scalar(out=q8[:, ch, ns], in0=qp,
                                            scalar1=A2[:, ch:ch + 1],
                                            scalar2=coef[:, ch:ch + 1],
                                            op0=MULT, op1=ADD)

        emit_q(0)

        # ---- rest of the loads (xb8 keys, weights for biases, xq last) ----
        for i in range(2):
            nc.sync.dma_start(out=xb8[:, i, NQ:N],
                              in_=xb8_d[:, N * i + NQ:N * (i + 1)])
        dxq = big.tile([128, 2, NQ], bf16)
        nc.sync.dma_start(out=dxq, in_=dxq_d)
        wall = consts.tile([128, 2, 2 * C], bf16)   # [wv, wp]
        nc.sync.dma_start(out=wall, in_=wall_d)
        pvt = consts.tile([128, 2, C], bf16)
        nc.sync.dma_start(out=pvt, in_=pvt_d)

        # ---- fold A into Pv^T -> fp8 (fused Wp @ Wv projection) ----
        pva8 = consts.tile([128, 2, C], f8)
        for i in range(2):
            nc.vector.tensor_scalar_mul(pva8[:, i, :], pvt[:, i, :],
                                        A2[:, i:i + 1])

        # ---- cv = Wv B + bv (bf16); bpe2 = bp + Wp cv; xqb = xq + bpe2 ----
        cvf = work.tile([128, 2], f32, name="cvf")
        for ot in range(2):
            cvpool = pw if ot == 0 else pr
            p = cvpool.tile([128, 1], f32, name="cvp",
                            tag="pw" if ot == 0 else "r")
            for i in range(2):
                nc.tensor.matmul(p, lhsT=wall[:, i, 128 * ot:128 * (ot + 1)],
                                 rhs=B2[:, i:i + 1], start=(i == 0), stop=(i == 1))
            nc.vector.tensor_sub(cvf[:, ot:ot + 1], bv2[:, ot:ot + 1], p)
        cv = consts.tile([128, 2], bf16)
        nc.vector.tensor_copy(cv, cvf)
        bpe2 = work.tile([128, 2], f32, name="bpe2")
        for ot in range(2):
            bppool = pw if ot == 0 else pr
            p2 = bppool.tile([128, 1], f32, name="bpp",
                             tag="pw" if ot == 0 else "r")
            for i in range(2):
                nc.tensor.matmul(p2, lhsT=wall[:, i, C + 128 * ot:C + 128 * (ot + 1)],
                                 rhs=cv[:, i:i + 1], start=(i == 0), stop=(i == 1))
            nc.vector.tensor_scalar_add(bpe2[:, ot:ot + 1], p2, bp2[:, ot:ot + 1])
        # residual x_q reconstructed as fp8(x) + bf16 quantization remainder
        xqb = big.tile([128, 2, NQ], f32)
        for ot in range(2):
            nc.gpsimd.tensor_add(xqb[:, ot, :], xb8[:, ot, 0:NQ], dxq[:, ot, :])
            nc.gpsimd.tensor_scalar_add(xqb[:, ot, :], xqb[:, ot, :],
                                        bpe2[:, ot:ot + 1])

        # ---- attention: S + delta -> exp -> r, u0, u1 -> scale -> project ----
        u8 = big.tile([128, 2, NQ], f8)
        for nch in range(2):
            ns = slice(nch * 512, (nch + 1) * 512)
            rp = pr.tile([128, 512], f32, name="rp", tag="r")
            up = [pacc.tile([128, 512], f32, name=f"up{h}", tag="u")
                  for h in range(2)]
            for s in range(16):
                if nch == 0 and s == 2:
                    emit_q(1, chans=(0,))
                if nch == 0 and s == 4:
                    emit_q(1, chans=(1,))
                sp = psp.tile([128, 1024], f32, name="sp", tag="sp")
                for h in range(2):
                    ms = slice((2 * s + h) * 128, (2 * s + h + 1) * 128)
                    hs = slice(512 * h, 512 * (h + 1))
                    nc.tensor.matmul(sp[:, hs], lhsT=xb8[:, :, ms],
                                     rhs=q8[:, :, ns], start=True, stop=True,
                                     perf_mode=DR)
                e = epool.tile([128, 2, 512], f8, name="e", tag="e")
                nc.scalar.activation(out=e, in_=sp, func=AF.Exp,
                                     scale=1.0 / 16.0, bias=nbias)
                nc.tensor.matmul(rp, lhsT=ones8, rhs=e,
                                 start=(s == 0), stop=(s == 15), perf_mode=DR)
                for h in range(2):
                    nc.tensor.matmul(
                        up[h],
                        lhsT=xt8[:, 2 * s:2 * s + 2, 136 * h:136 * h + 128],
                        rhs=e, start=(s == 0), stop=(s == 15), perf_mode=DR)
            rb = work.tile([128, 512], f32, name="rb", tag="rb", bufs=2)
            nc.vector.reciprocal(out=rb, in_=rp)
            for h in range(2):
                nc.vector.tensor_mul(u8[:, h, ns], up[h], rb)

            # fused projection; out = pp/64 + (xq + bpe) in one stt
            for ot in range(2):
                if nch == 1:
                    pp = psp.tile([128, 512], f32, name="pp", tag="sp")
                else:
                    pp = pw.tile([128, 512], f32, name="pp", tag="pw")
                nc.tensor.matmul(pp, lhsT=pva8[:, :, ot * 128:(ot + 1) * 128],
                                 rhs=u8[:, :, ns], start=True, stop=True,
                                 perf_mode=DR)
                ot_t = work.tile([128, 512], bf16, name="ot_t", tag="ot_t")
                nc.vector.scalar_tensor_tensor(out=ot_t, in0=pp,
                                               scalar=1.0 / 64.0,
                                               in1=xqb[:, ot, ns], op0=MULT,
                                               op1=ADD)
                nc.sync.dma_start(out=out_d[:, NQ * ot + 512 * nch:
                                            NQ * ot + 512 * (nch + 1)],
                                  in_=ot_t)

    nc.compile()
    return nc


def _get_nc():
    key = "nc"
    if key not in _CACHE:
        _CACHE[key] = _build_nc()
    return _CACHE[key]


def _host_inputs(x, gamma, beta, Wq, bq, Wk, bk, Wv, bv, Wp, bp):
    import ml_dtypes
    f8 = ml_dtypes.float8_e4m3
    bf = ml_dtypes.bfloat16

    x = np.asarray(x, np.float32)
    xf = np.ascontiguousarray(x.reshape(2, C, N))
    gamma = np.asarray(gamma, np.float32)
    beta = np.asarray(beta, np.float32)
    Wq, Wk, Wv, Wp = [np.asarray(W, np.float32) for W in (Wq, Wk, Wv, Wp)]
    bq, bv, bp = [np.asarray(v, np.float32) for v in (bq, bv, bp)]

    # host-fused matrices: G = Wk^T Wq, Pv = Wp @ Wv
    G = Wk.T @ Wq                                             # [c, c']
    Pv = Wp @ Wv                                              # [o, c]
    # gt[p, i, c] = G[c, i*128+p]  (contraction over c' = i*128+p)
    gt = np.ascontiguousarray(
        G.T.reshape(2, 128, C).transpose(1, 0, 2).reshape(128, 2 * C)
    ).astype(bf)
    # pvt[p, i, o] = Pv[o, i*128+p]; the r-matmul's 1/64 stationary makes
    # u8 = 64*u/r (fp8 normal range), repaid by the epilogue's /64.
    pvt = np.ascontiguousarray(
        Pv.T.reshape(2, 128, C).transpose(1, 0, 2).reshape(128, 2 * C)
    ).astype(bf)
    # wall[p, i, (wv|wp), o] = W[o, i*128+p]
    wall = np.stack([Wv.T, Wp.T], axis=1)                     # [c, 2, o]
    wall = wall.reshape(2, 128, 2, C).transpose(1, 0, 2, 3)
    wall = np.ascontiguousarray(wall.reshape(128, 2 * 2 * C)).astype(bf)

    small = np.zeros((128, 26), np.float32)
    small[:, 0:2] = gamma.reshape(2, 128).T
    small[:, 2:4] = beta.reshape(2, 128).T
    small[:, 4:6] = bv.reshape(2, 128).T
    small[:, 6:8] = (Wk.T @ bq).reshape(2, 128).T
    small[:, 8:10] = bp.reshape(2, 128).T
    cids = np.arange(C)
    gm = np.zeros((C, GROUPS), np.float32)
    gm[cids, cids // GSIZE] = 1.0                             # [c, g]
    gmT = gm.reshape(2, 128, GROUPS).transpose(1, 0, 2)       # [p, i, g]
    small[:, 10:18] = gmT[:, 0, :] / (GSIZE * N)
    small[:, 18:26] = gmT[:, 1, :] / (GSIZE * N)
    gmask8 = np.ascontiguousarray(gm.T)                       # [g, c]
    imask = np.eye(128, dtype=np.float32).astype(bf)

    in_maps = []
    for core in range(8):
        b, j = divmod(core, 4)
        xrot = np.roll(xf[b], -j * NQ, axis=1)                # [C, N]
        x8 = xrot.astype(f8)
        xb8 = np.ascontiguousarray(
            x8.reshape(2, 128, N).transpose(1, 0, 2).reshape(128, 2 * N))
        dxq = (xrot[:, :NQ] - x8[:, :NQ].astype(np.float32))
        dxq = np.ascontiguousarray(
            dxq.reshape(2, 128, NQ).transpose(1, 0, 2).reshape(128, 2 * NQ)
        ).astype(bf)
        # xt8: position-major with ones cols: per n: [c0..c127, 1, c128.., 1]
        xt = x8.astype(np.float32).T                          # [n, c] quantized
        arr = np.zeros((N, 272), np.float32)
        arr[:, 0:128] = xt[:, 0:128]
        arr[:, 128] = 1.0
        arr[:, 136:264] = xt[:, 128:256]
        arr[:, 264] = 1.0
        xt8 = np.ascontiguousarray(
            arr.reshape(32, 128, 272).transpose(1, 0, 2).reshape(128, 32 * 272)
        ).astype(f8)
        in_maps.append({
            "xt8": xt8, "xb8": xb8, "gt": gt, "pvt": pvt, "wall": wall,
            "small": small, "gmask8": gmask8, "imask": imask,
            "dxq": dxq,
        })
    return in_maps


def kernel(x, gamma, beta, Wq, bq, Wk, bk, Wv, bv, Wp, bp):
    from concourse.bass_utils import run_bass_kernel_spmd
    global LAST_RESULTS

    orig_shape = np.asarray(x).shape
    in_maps = _host_inputs(x, gamma, beta, Wq, bq, Wk, bk, Wv, bv, Wp, bp)
    nc = _get_nc()

    trace = os.environ.get("BASSK_TRACE", "0") == "1"
    res = run_bass_kernel_spmd(nc, in_maps, core_ids=list(range(8)), trace=trace)
    LAST_RESULTS = res

    out = np.empty((2, C, N), np.float32)
    for core in range(8):
        b, j = divmod(core, 4)
        o = res.results[core]["out"].astype(np.float32)       # [128, 2*NQ]
        o = o.reshape(128, 2, NQ).transpose(1, 0, 2).reshape(C, NQ)
        out[b][:, j * NQ:(j + 1) * NQ] = o
    return out.reshape(orig_shape)
